# revision 13
# baseline (speedup 1.0000x reference)
"""Trainium2 Bass kernel for MHSA with relative position bias (nn_MHSARPB).

Problem (hardcoded): x (8, 32, 32, 512), qkv_w (1536, 512), qkv_b (1536,),
rpb (16, 63, 63), proj_w (512, 512), proj_b (512,). Output (8, 32, 32, 512) f32.

Sharding: 4 heads x 4 batches per core (8 cores = 4 head-groups x 2 batch
groups). Each core computes q/k/v for its 4 heads over its 4 batches (4096
tokens), full attention for those (4 batch x 4 head) pairs, and a partial
projection (contraction over its 128 channels). Host sums the 4 head-group
partials per batch-group and adds proj_b.

Per-core dataflow (all fp16 on SBUF, fp32 in PSUM):
  - qkv:   qT/kT/vT = W_h @ x^T per 512-token chunk, bias added during the
           PSUM->SBUF evacuation (tensor_scalar_add). q pre-scaled on host.
  - v_nat: per (batch, key-chunk) [128 keys, 128 dims] via dma_start_transpose.
  - S^T:   [128 keys, 512 queries] per (b, half, j, head) via 4-way row-packed
           K=32 matmuls (tile_position=(32*hh, 0)) reading qT/kT in natural
           layout -- no kT_pack / q replication needed with 4 heads/core.
  - softmax: E = exp(S^T) * exp(bias^T); exp on ScalarE (PSUM->SBUF fp16,
           no max-subtraction: |logits| <= ~3), bias factor from a compressed
           per-partition-shifted table (tabR, [128, 4*60*32]) applied as one
           strided DVE multiply per S tile. Denominators via col-tiled M=1
           ones-matmuls accumulating next to the AV matmuls.
  - AV:    Y[128 dims, 512 q] accumulated over 8 key-chunks, 4 heads
           col-packed (tile_position=(0, 32*hh)).
  - norm:  r = 1/den broadcast across each head's 32 partitions via a DRAM
           bounce, y_norm = y * r on DVE.
  - proj:  partial P[512, T'] = projT.T @ y_norm, K=128 (all rows useful).

qkv/v-transpose/proj emission is woven into the attention j-loop of the
previous/next batch so ScalarE (the exp stream, the bottleneck engine at
~1.1us per [128,1024] tile) never starves.
"""
import sys

sys.path.insert(0, "/opt/trn_rl_repo")

import contextlib
import numpy as np
import concourse.bass as bass
import concourse.bacc as bacc
import concourse.tile as tile
from concourse import mybir
from concourse.bass_utils import run_bass_kernel_spmd

FP16 = mybir.dt.float16
FP32 = mybir.dt.float32
EXP = mybir.ActivationFunctionType.Exp

B, S, C, NH = 8, 32, 512, 16
N = S * S            # 1024 tokens per image
D = C // NH          # 32 head dim
SCALE = D ** -0.5
N_CORES = 8
NB = 4               # batches per core
TC = NB * N          # 4096 tokens per core

# Softmax path per (half, j) slot: "A" = ACT exp + DVE bias-mul, "B" = ACT
# exp + GpSimd bias-mul, "C" = fused Schraudolph exp(S+bias) on DVE
# (scalar_tensor_tensor -> int16 bit-trick -> bitcast fp16, ~1.8% elementwise
# noise that washes out in the softmax average; end-to-end rel err ~5e-3).
PATTERN = "ACBACBACBACBACBA"
A16 = 1024.0 / float(np.log(2.0))
B16 = 15.0 * 1024.0 - 60.0

_CACHE = {}


def build_nc(repeat=1):
    nc = bacc.Bacc("TRN2", target_bir_lowering=False, debug=False)

    xT = nc.dram_tensor("xT", [4, 128, TC], FP16, kind="ExternalInput")
    wq = nc.dram_tensor("wq", [4, 128, 128], FP16, kind="ExternalInput")
    wk = nc.dram_tensor("wk", [4, 128, 128], FP16, kind="ExternalInput")
    wv = nc.dram_tensor("wv", [4, 128, 128], FP16, kind="ExternalInput")
    bq = nc.dram_tensor("bq", [128, 1], FP32, kind="ExternalInput")
    bk = nc.dram_tensor("bk", [128, 1], FP32, kind="ExternalInput")
    bv = nc.dram_tensor("bv", [128, 1], FP32, kind="ExternalInput")
    tabR = nc.dram_tensor("tabR", [128, 4 * 60 * 32], FP16, kind="ExternalInput")
    tabL = nc.dram_tensor("tabL", [128, 4 * 60 * 32], FP16, kind="ExternalInput")
    projT = nc.dram_tensor("projT", [128, 512], FP16, kind="ExternalInput")
    outT = nc.dram_tensor("outT", [C, TC], FP16, kind="ExternalOutput")
    den_scr = nc.dram_tensor("den_scr", [16, N], FP16)

    with tile.TileContext(nc) as tc:
        with (
            tc.For_i(0, repeat, 1) if repeat > 1 else contextlib.nullcontext(),
            tc.tile_pool(name="consts", bufs=1) as consts,
            tc.tile_pool(name="big", bufs=1) as big,
            tc.tile_pool(name="xin", bufs=2) as xin,
            tc.tile_pool(name="vstg", bufs=2) as vstg,
            tc.tile_pool(name="vnat", bufs=2) as vnat,
            tc.tile_pool(name="epool", bufs=4) as epool,
            tc.tile_pool(name="e2pool", bufs=4) as e2pool,
            tc.tile_pool(name="ypool", bufs=2) as ypool,
            tc.tile_pool(name="rpool", bufs=2) as rpool,
            tc.tile_pool(name="opool", bufs=2) as opool,
        ):
            # ---- constants -------------------------------------------------
            wq_sb = consts.tile([128, 4 * 128], FP16, tag="wq_sb")
            nc.sync.dma_start(
                out=wq_sb.rearrange("p (kc f) -> p kc f", kc=4),
                in_=wq[:].transpose([1, 0, 2]),
            )
            wk_sb = consts.tile([128, 4 * 128], FP16, tag="wk_sb")
            nc.sync.dma_start(
                out=wk_sb.rearrange("p (kc f) -> p kc f", kc=4),
                in_=wk[:].transpose([1, 0, 2]),
            )
            wv_sb = consts.tile([128, 4 * 128], FP16, tag="wv_sb")
            nc.sync.dma_start(
                out=wv_sb.rearrange("p (kc f) -> p kc f", kc=4),
                in_=wv[:].transpose([1, 0, 2]),
            )
            bq_sb = consts.tile([128, 1], FP32, tag="bq_sb")
            nc.sync.dma_start(out=bq_sb[:], in_=bq[:])
            bk_sb = consts.tile([128, 1], FP32, tag="bk_sb")
            nc.sync.dma_start(out=bk_sb[:], in_=bk[:])
            bv_sb = consts.tile([128, 1], FP32, tag="bv_sb")
            nc.sync.dma_start(out=bv_sb[:], in_=bv[:])
            tab_sb = consts.tile([128, 4 * 60 * 32], FP16, tag="tab_sb")
            nc.sync.dma_start(out=tab_sb[:], in_=tabR[:])
            tabL_sb = consts.tile([128, 4 * 60 * 32], FP16, tag="tabL_sb")
            nc.sync.dma_start(out=tabL_sb[:], in_=tabL[:])
            projT_sb = consts.tile([128, 512], FP16, tag="projT_sb")
            nc.sync.dma_start(out=projT_sb[:], in_=projT[:])
            ones_sb = consts.tile([128, 1], FP16, tag="ones_sb")
            nc.gpsimd.memset(ones_sb[:], 1.0)

            tab4 = tab_sb.rearrange("p (h y j) -> p h y j", h=4, y=60)
            tabL4 = tabL_sb.rearrange("p (h y j) -> p h y j", h=4, y=60)

            # ---- persistent big tensors -----------------------------------
            qT_sb = big.tile([128, TC], FP16, tag="qT_sb")
            kT_sb = big.tile([128, TC], FP16, tag="kT_sb")

            with (
                tc.tile_pool(name="ps_s", bufs=2, space="PSUM") as ps_s,
                tc.tile_pool(name="ps_av", bufs=1, space="PSUM") as ps_av,
                tc.tile_pool(name="ps_den", bufs=1, space="PSUM") as ps_den,
                tc.tile_pool(name="ps_pj", bufs=1, space="PSUM") as ps_pj,
                tc.tile_pool(name="ps_qkv", bufs=1, space="PSUM") as ps_qkv,
            ):
                state = {}

                def qkv_units(b):
                    """Emission units (closures) producing qT/kT/vT/v_nat for
                    batch b. Each unit is ~1-2.5us of PE/DMA work."""
                    units = []
                    xts = {}
                    vT_stg = vstg.tile([128, 1024], FP16, tag="vstg")
                    v_nat = vnat.tile([128, 1024], FP16, tag="vnat")
                    state[("vnat", b)] = v_nat

                    def load_xt(cc):
                        def f():
                            xt = xin.tile([128, 2048], FP16, tag="xt")
                            c = 2 * b + cc
                            nc.sync.dma_start(
                                out=xt.rearrange("p (kc f) -> p kc f", kc=4),
                                in_=xT[:].transpose([1, 0, 2])[
                                    :, :, c * 512 : (c + 1) * 512
                                ],
                            )
                            xts[cc] = xt
                        return f

                    def mm_part(cc, w_sb, b_sb, dst):
                        def f():
                            ps = ps_qkv.tile([128, 512], FP32, tag="qkv")
                            xt = xts[cc]
                            for kc in range(4):
                                nc.tensor.matmul(
                                    ps[:],
                                    w_sb[:, kc * 128 : (kc + 1) * 128],
                                    xt[:, kc * 512 : (kc + 1) * 512],
                                    start=(kc == 0), stop=(kc == 3),
                                )
                            nc.vector.tensor_scalar_add(dst, ps[:], b_sb[:])
                        return f

                    def v_transp(jlo, jhi):
                        def f():
                            for j in range(jlo, jhi):
                                nc.sync.dma_start_transpose(
                                    out=v_nat[:, j * 128 : (j + 1) * 128],
                                    in_=vT_stg[:, j * 128 : (j + 1) * 128],
                                )
                        return f

                    for cc in range(2):
                        c = 2 * b + cc
                        units.append(load_xt(cc))
                        units.append(mm_part(
                            cc, wq_sb, bq_sb, qT_sb[:, c * 512 : (c + 1) * 512]))
                        units.append(mm_part(
                            cc, wk_sb, bk_sb, kT_sb[:, c * 512 : (c + 1) * 512]))
                        units.append(mm_part(
                            cc, wv_sb, bv_sb, vT_stg[:, cc * 512 : (cc + 1) * 512]))
                        units.append(v_transp(4 * cc, 4 * cc + 4))
                    return units

                def proj_units(b):
                    """Normalize + project + store batch b (y_sb/r ready)."""
                    units = []
                    y_sb = state.pop(("y", b))
                    r_sb = state.pop(("r", b))

                    def r_store():
                        for hh in range(4):
                            nc.gpsimd.dma_start(
                                out=den_scr[b * 4 + hh : b * 4 + hh + 1, :],
                                in_=r_sb[32 * hh : 32 * hh + 1, :],
                            )
                    units.append(r_store)

                    r_bc = rpool.tile([128, 1024], FP16, tag="r_bc")

                    def r_load():
                        for hh in range(4):
                            nc.gpsimd.dma_start(
                                out=r_bc[32 * hh : 32 * hh + 32, :],
                                in_=bass.AP(
                                    tensor=den_scr,
                                    offset=(b * 4 + hh) * N,
                                    ap=[[0, 32], [1, N]],
                                ),
                            )
                    units.append(r_load)

                    y_n = ypool.tile([128, 1024], FP16, tag="y_n")

                    def norm():
                        nc.vector.tensor_mul(y_n[:], y_sb[:], r_bc[:])
                    units.append(norm)

                    def pj_cs(cs):
                        def f():
                            o_t = opool.tile([128, 1024], FP16, tag="o_t")
                            for half in range(2):
                                pj = ps_pj.tile([128, 512], FP32, tag="pj")
                                nc.tensor.matmul(
                                    pj[:],
                                    projT_sb[:, cs * 128 : (cs + 1) * 128],
                                    y_n[:, half * 512 : (half + 1) * 512],
                                    start=True, stop=True,
                                )
                                nc.vector.tensor_copy(
                                    o_t[:, half * 512 : (half + 1) * 512], pj[:])
                            nc.sync.dma_start(
                                out=outT[cs * 128 : (cs + 1) * 128,
                                         b * 1024 : (b + 1) * 1024],
                                in_=o_t[:],
                            )
                        return f
                    for cs in range(4):
                        units.append(pj_cs(cs))
                    return units

                def attn_batch(b, woven):
                    """Attention for batch b; `woven` units interleaved into
                    the j-loop to keep PE/DMA fed without starving ACT."""
                    v_nat = state[("vnat", b)]
                    y_sb = ypool.tile([128, 1024], FP16, tag="y_sb")
                    r_sb = rpool.tile([128, 1024], FP16, tag="r_sb")
                    state[("y", b)] = y_sb
                    state[("r", b)] = r_sb
                    wi = 0
                    for half in range(2):
                        av = ps_av.tile([128, 512], FP32, tag="av")
                        den = ps_den.tile([128, 512], FP32, tag="den")

                        def av_den(j, e2s):
                            # AV + den, 4 heads col-packed
                            for hh in range(4):
                                nc.tensor.matmul(
                                    av[32 * hh : 32 * hh + 32, :],
                                    v_nat[:, j * 128 + 32 * hh :
                                          j * 128 + 32 * hh + 32],
                                    e2s[hh // 2][:, (hh % 2) * 512 :
                                                 (hh % 2) * 512 + 512],
                                    start=(j == 0), stop=(j == 7),
                                    tile_position=(0, 32 * hh),
                                )
                            for hh in range(4):
                                nc.tensor.matmul(
                                    den[32 * hh : 32 * hh + 1, :],
                                    ones_sb[:],
                                    e2s[hh // 2][:, (hh % 2) * 512 :
                                                 (hh % 2) * 512 + 512],
                                    start=(j == 0), stop=(j == 7),
                                    tile_position=(0, 32 * hh),
                                )

                        prev = None
                        for j in range(8):
                            # scores: 4-way row-packed K=32 matmuls
                            s_ts = []
                            for hp in range(2):
                                sps = ps_s.tile([128, 1024], FP32, tag="sps")
                                for h2 in range(2):
                                    hh = 2 * hp + h2
                                    nc.tensor.matmul(
                                        sps[:, h2 * 512 : (h2 + 1) * 512],
                                        kT_sb[32 * hh : 32 * hh + 32,
                                              b * 1024 + j * 128 :
                                              b * 1024 + j * 128 + 128],
                                        qT_sb[32 * hh : 32 * hh + 32,
                                              b * 1024 + half * 512 :
                                              b * 1024 + half * 512 + 512],
                                        start=True, stop=True,
                                        tile_position=(32 * hh, 0),
                                    )
                                s_ts.append(sps)
                            # exp + bias multiply (path per PATTERN slot)
                            path = PATTERN[half * 8 + j]
                            e2s = []
                            yr0 = 28 - 4 * j + 16 * half
                            for hp in range(2):
                                if path == "C":
                                    # fused exp(S+bias) via int16 bit-trick
                                    e2i = e2pool.tile([128, 1024],
                                                      mybir.dt.int16,
                                                      tag="E2i")
                                    nc.vector.scalar_tensor_tensor(
                                        e2i.rearrange(
                                            "p (h a b) -> p h a b", h=2, a=16),
                                        s_ts[hp].rearrange(
                                            "p (h a b) -> p h a b", h=2, a=16),
                                        A16,
                                        tabL4[:, 2 * hp : 2 * hp + 2,
                                              yr0 : yr0 + 16, :],
                                        mybir.AluOpType.mult,
                                        mybir.AluOpType.add,
                                    )
                                    e2s.append(e2i.bitcast(FP16))
                                    continue
                                e_t = epool.tile([128, 1024], FP16, tag="E")
                                nc.scalar.activation(e_t[:], s_ts[hp][:], EXP)
                                e2 = e2pool.tile([128, 1024], FP16, tag="E2")
                                eng = nc.gpsimd if path == "B" else nc.vector
                                eng.tensor_mul(
                                    e2.rearrange("p (h a b) -> p h a b",
                                                 h=2, a=16),
                                    e_t.rearrange("p (h a b) -> p h a b",
                                                  h=2, a=16),
                                    tab4[:, 2 * hp : 2 * hp + 2,
                                         yr0 : yr0 + 16, :],
                                )
                                e2s.append(e2)
                            # lag-1: AV/den for j-1 emit after scores(j) so
                            # the in-order PE queue issues scores(j) without
                            # waiting on the DVE bias-multiply of j-1
                            if prev is not None:
                                av_den(*prev)
                            prev = (j, e2s)
                            if wi < len(woven) and \
                                    wi * 16 < (j + 1 + half * 8) * len(woven):
                                woven[wi]()
                                wi += 1
                        av_den(*prev)
                        nc.vector.tensor_copy(
                            y_sb[:, half * 512 : (half + 1) * 512], av[:])
                        with nc.allow_low_precision(
                                reason="1/den in fp16: den~1e3, 5e-4 rel ok"):
                            nc.vector.reciprocal(
                                r_sb[:, half * 512 : (half + 1) * 512], den[:])
                    while wi < len(woven):
                        woven[wi]()
                        wi += 1

                # ---- schedule -------------------------------------------
                for u in qkv_units(0):
                    u()
                for b in range(NB):
                    woven = []
                    if b + 1 < NB:
                        woven += qkv_units(b + 1)
                    if b - 1 >= 0:
                        woven += proj_units(b - 1)
                    attn_batch(b, woven)
                for u in proj_units(NB - 1):
                    u()
    nc.compile()
    return nc


def _prep_inputs(x, qkv_w, qkv_b, rpb, proj_w, proj_b):
    x = np.asarray(x, np.float32)
    qkv_w = np.asarray(qkv_w, np.float32)
    qkv_b = np.asarray(qkv_b, np.float32)
    rpb = np.asarray(rpb, np.float32)
    proj_w = np.asarray(proj_w, np.float32)

    p = np.arange(128)
    pm, pd = p % 32, p // 32
    yr = np.arange(60)
    j1 = np.arange(32)
    row = 59 - yr[None, :, None] + pd[:, None, None]      # (128, 60, 1)
    col = 31 + pm[:, None, None] - j1[None, None, :]      # (128, 1, 32)
    row_b = np.broadcast_to(row, (128, 60, 32))
    col_b = np.broadcast_to(col, (128, 60, 32))

    in_maps = []
    for core in range(N_CORES):
        hg, bg = core // 2, core % 2
        xs = x[4 * bg : 4 * bg + 4].reshape(TC, C)
        xT16 = np.ascontiguousarray(xs.T).astype(np.float16).reshape(4, 128, TC)

        r0 = 128 * hg
        wq_in = (qkv_w[r0 : r0 + 128, :] * SCALE).T.reshape(4, 128, 128)
        wk_in = qkv_w[C + r0 : C + r0 + 128, :].T.reshape(4, 128, 128)
        wv_in = qkv_w[2 * C + r0 : 2 * C + r0 + 128, :].T.reshape(4, 128, 128)
        bq_in = (qkv_b[r0 : r0 + 128] * SCALE).astype(np.float32).reshape(128, 1)
        bk_in = qkv_b[C + r0 : C + r0 + 128].astype(np.float32).reshape(128, 1)
        bv_in = qkv_b[2 * C + r0 : 2 * C + r0 + 128].astype(np.float32).reshape(128, 1)

        tabs, tabsL = [], []
        for hh in range(4):
            e = np.exp(rpb[4 * hg + hh])[row_b, col_b]    # (128, 60, 32)
            tabs.append(e.reshape(128, 1920))
            el = (A16 * rpb[4 * hg + hh] + B16)[row_b, col_b]
            tabsL.append(el.reshape(128, 1920))
        tab_in = np.concatenate(tabs, axis=1).astype(np.float16)
        tabL_in = np.concatenate(tabsL, axis=1).astype(np.float16)

        projT_in = np.ascontiguousarray(
            proj_w[:, r0 : r0 + 128].T).astype(np.float16)

        in_maps.append({
            "xT": xT16,
            "wq": np.ascontiguousarray(wq_in).astype(np.float16),
            "wk": np.ascontiguousarray(wk_in).astype(np.float16),
            "wv": np.ascontiguousarray(wv_in).astype(np.float16),
            "bq": bq_in, "bk": bk_in, "bv": bv_in,
            "tabR": tab_in,
            "tabL": tabL_in,
            "projT": projT_in,
        })
    return in_maps


def kernel(x, qkv_w, qkv_b, rpb, proj_w, proj_b):
    if "nc" not in _CACHE:
        _CACHE["nc"] = build_nc()
    nc = _CACHE["nc"]
    in_maps = _prep_inputs(x, qkv_w, qkv_b, rpb, proj_w, proj_b)
    res = run_bass_kernel_spmd(nc, in_maps, list(range(N_CORES)))
    out = np.zeros((B * N, C), np.float32)
    for core in range(N_CORES):
        hg, bg = core // 2, core % 2
        pt = res.results[core]["outT"].astype(np.float32)   # (512, 4096)
        out[4 * bg * N : (4 * bg + 4) * N, :] += pt.T
    out += np.asarray(proj_b, np.float32)[None, :]
    return out.reshape(B, S, S, C)


if __name__ == "__main__":
    rng = np.random.default_rng(0)
    ins = {
        "x": rng.standard_normal((B, S, S, C)).astype(np.float32),
        "qkv_w": (rng.standard_normal((3 * C, C)) * 0.02).astype(np.float32),
        "qkv_b": (rng.standard_normal((3 * C,)) * 0.02).astype(np.float32),
        "rpb": (rng.standard_normal((NH, 2 * S - 1, 2 * S - 1)) * 0.02).astype(np.float32),
        "proj_w": (rng.standard_normal((C, C)) * 0.02).astype(np.float32),
        "proj_b": (rng.standard_normal((C,)) * 0.02).astype(np.float32),
    }
    out = kernel(**ins)
    print("kernel ran, out", out.shape, out.dtype, float(np.abs(out).max()))


# revision 26
# speedup vs baseline: 2.4556x; 2.4556x over previous
"""Trainium2 Bass kernel for MHSA with relative position bias (nn_MHSARPB).

Problem (hardcoded): x (8, 32, 32, 512), qkv_w (1536, 512), qkv_b (1536,),
rpb (16, 63, 63), proj_w (512, 512), proj_b (512,). Output (8, 32, 32, 512) f32.

Sharding: 4 heads x 4 batches per core (8 cores = 4 head-groups x 2 batch
groups). Each core computes q/k/v for its 4 heads over its 4 batches (4096
tokens), full attention for those (4 batch x 4 head) pairs, and a partial
projection (contraction over its 128 channels). Host sums the 4 head-group
partials per batch-group and adds proj_b.

Per-core dataflow (all fp16 on SBUF, fp32 in PSUM):
  - qkv:   qT/kT/vT = W_h @ x^T per 512-token chunk, bias added during the
           PSUM->SBUF evacuation (tensor_scalar_add). q pre-scaled on host.
  - v_nat: per (batch, key-chunk) [128 keys, 128 dims] via dma_start_transpose.
  - S^T:   [128 keys, 512 queries] per (b, half, j, head) via 4-way row-packed
           K=32 matmuls (tile_position=(32*hh, 0)) reading qT/kT in natural
           layout -- no kT_pack / q replication needed with 4 heads/core.
  - softmax: E = exp(S^T) * exp(bias^T); exp on ScalarE (PSUM->SBUF fp16,
           no max-subtraction: |logits| <= ~3), bias factor from a compressed
           per-partition-shifted table (tabR, [128, 4*60*32]) applied as one
           strided DVE multiply per S tile. Denominators via col-tiled M=1
           ones-matmuls accumulating next to the AV matmuls.
  - AV:    Y[128 dims, 512 q] accumulated over 8 key-chunks, 4 heads
           col-packed (tile_position=(0, 32*hh)).
  - norm:  r = 1/den broadcast across each head's 32 partitions via a DRAM
           bounce, y_norm = y * r on DVE.
  - proj:  partial P[512, T'] = projT.T @ y_norm, K=128 (all rows useful).

qkv/v-transpose/proj emission is woven into the attention j-loop of the
previous/next batch so ScalarE (the exp stream, the bottleneck engine at
~1.1us per [128,1024] tile) never starves.
"""
import sys

sys.path.insert(0, "/opt/trn_rl_repo")

import contextlib
import numpy as np
import concourse.bass as bass
import concourse.bacc as bacc
import concourse.tile as tile
from concourse import mybir
from concourse.bass_utils import run_bass_kernel_spmd

FP16 = mybir.dt.float16
FP32 = mybir.dt.float32
EXP = mybir.ActivationFunctionType.Exp

B, S, C, NH = 8, 32, 512, 16
N = S * S            # 1024 tokens per image
D = C // NH          # 32 head dim
SCALE = D ** -0.5
N_CORES = 8
NB = 4               # batches per core
TC = NB * N          # 4096 tokens per core

# Softmax path per (half, j) slot: "A" = ACT exp + DVE bias-mul, "B" = ACT
# exp + GpSimd bias-mul, "C" = fused Schraudolph exp(S+bias) on DVE
# (scalar_tensor_tensor -> int16 bit-trick -> bitcast fp16, ~1.8% elementwise
# noise that washes out in the softmax average; end-to-end rel err ~5e-3),
# "D" = bias added in PSUM by PE diagonal-identity matmuls (emitted before
# the scores matmuls, pipelining into the spare S buffer during the previous
# exp) + plain ACT exp.
PATTERN = "CCDCCDCCDCDCCDCD"
A16 = 1024.0 / float(np.log(2.0))
B16 = 15.0 * 1024.0 - 60.0

_CACHE = {}


def build_nc(repeat=1):
    nc = bacc.Bacc("TRN2", target_bir_lowering=False, debug=False)

    xT = nc.dram_tensor("xT", [4, 128, TC], FP16, kind="ExternalInput")
    wq = nc.dram_tensor("wq", [4, 128, 128], FP16, kind="ExternalInput")
    wk = nc.dram_tensor("wk", [4, 128, 128], FP16, kind="ExternalInput")
    wv = nc.dram_tensor("wv", [4, 128, 128], FP16, kind="ExternalInput")
    bq = nc.dram_tensor("bq", [128, 1], FP32, kind="ExternalInput")
    bk = nc.dram_tensor("bk", [128, 1], FP32, kind="ExternalInput")
    bv = nc.dram_tensor("bv", [128, 1], FP32, kind="ExternalInput")
    tabR = nc.dram_tensor("tabR", [128, 4 * 60 * 32], FP16, kind="ExternalInput")
    tabL = nc.dram_tensor("tabL", [128, 4 * 60 * 32], FP16, kind="ExternalInput")
    tabD = nc.dram_tensor("tabD", [128, 4 * 60 * 32], FP16, kind="ExternalInput")
    ident = nc.dram_tensor("ident", [128, 32], FP16, kind="ExternalInput")
    projT = nc.dram_tensor("projT", [128, 512], FP16, kind="ExternalInput")
    outT = nc.dram_tensor("outT", [C, TC], FP16, kind="ExternalOutput")
    den_scr = nc.dram_tensor("den_scr", [16, N], FP16)

    with tile.TileContext(nc) as tc:
        with (
            tc.For_i(0, repeat, 1) if repeat > 1 else contextlib.nullcontext(),
            tc.tile_pool(name="consts", bufs=1) as consts,
            tc.tile_pool(name="big", bufs=1) as big,
            tc.tile_pool(name="xin", bufs=2) as xin,
            tc.tile_pool(name="vstg", bufs=2) as vstg,
            tc.tile_pool(name="vnat", bufs=2) as vnat,
            tc.tile_pool(name="epool", bufs=4) as epool,
            tc.tile_pool(name="e2pool", bufs=4) as e2pool,
            tc.tile_pool(name="ypool", bufs=2) as ypool,
            tc.tile_pool(name="rpool", bufs=2) as rpool,
            tc.tile_pool(name="opool", bufs=2) as opool,
        ):
            # ---- constants -------------------------------------------------
            wq_sb = consts.tile([128, 4 * 128], FP16, tag="wq_sb")
            nc.sync.dma_start(
                out=wq_sb.rearrange("p (kc f) -> p kc f", kc=4),
                in_=wq[:].transpose([1, 0, 2]),
            )
            wk_sb = consts.tile([128, 4 * 128], FP16, tag="wk_sb")
            nc.sync.dma_start(
                out=wk_sb.rearrange("p (kc f) -> p kc f", kc=4),
                in_=wk[:].transpose([1, 0, 2]),
            )
            wv_sb = consts.tile([128, 4 * 128], FP16, tag="wv_sb")
            nc.sync.dma_start(
                out=wv_sb.rearrange("p (kc f) -> p kc f", kc=4),
                in_=wv[:].transpose([1, 0, 2]),
            )
            bq_sb = consts.tile([128, 1], FP32, tag="bq_sb")
            nc.sync.dma_start(out=bq_sb[:], in_=bq[:])
            bk_sb = consts.tile([128, 1], FP32, tag="bk_sb")
            nc.sync.dma_start(out=bk_sb[:], in_=bk[:])
            bv_sb = consts.tile([128, 1], FP32, tag="bv_sb")
            nc.sync.dma_start(out=bv_sb[:], in_=bv[:])
            tab_sb = consts.tile([128, 4 * 60 * 32], FP16, tag="tab_sb")
            nc.sync.dma_start(out=tab_sb[:], in_=tabR[:])
            tabL_sb = consts.tile([128, 4 * 60 * 32], FP16, tag="tabL_sb")
            nc.sync.dma_start(out=tabL_sb[:], in_=tabL[:])
            tabD_sb = consts.tile([128, 4 * 60 * 32], FP16, tag="tabD_sb")
            nc.sync.dma_start(out=tabD_sb[:], in_=tabD[:])
            ident_sb = consts.tile([128, 32], FP16, tag="ident_sb")
            nc.sync.dma_start(out=ident_sb[:], in_=ident[:])
            projT_sb = consts.tile([128, 512], FP16, tag="projT_sb")
            nc.sync.dma_start(out=projT_sb[:], in_=projT[:])
            ones_sb = consts.tile([128, 1], FP16, tag="ones_sb")
            nc.gpsimd.memset(ones_sb[:], 1.0)

            tab4 = tab_sb.rearrange("p (h y j) -> p h y j", h=4, y=60)
            tabL4 = tabL_sb.rearrange("p (h y j) -> p h y j", h=4, y=60)
            tabD4 = tabD_sb.rearrange("p (h y j) -> p h y j", h=4, y=60)

            # ---- persistent big tensors -----------------------------------
            qT_sb = big.tile([128, TC], FP16, tag="qT_sb")
            kT_sb = big.tile([128, TC], FP16, tag="kT_sb")

            with (
                tc.tile_pool(name="ps_s", bufs=2, space="PSUM") as ps_s,
                tc.tile_pool(name="ps_av", bufs=1, space="PSUM") as ps_av,
                tc.tile_pool(name="ps_den", bufs=1, space="PSUM") as ps_den,
                tc.tile_pool(name="ps_pj", bufs=1, space="PSUM") as ps_pj,
                tc.tile_pool(name="ps_qkv", bufs=1, space="PSUM") as ps_qkv,
            ):
                state = {}

                def qkv_units(b):
                    """Emission units (closures) producing qT/kT/vT/v_nat for
                    batch b. Each unit is ~1-2.5us of PE/DMA work."""
                    units = []
                    xts = {}
                    vT_stg = vstg.tile([128, 1024], FP16, tag="vstg")
                    v_nat = vnat.tile([128, 1024], FP16, tag="vnat")
                    state[("vnat", b)] = v_nat

                    def load_xt(cc):
                        def f():
                            xt = xin.tile([128, 2048], FP16, tag="xt")
                            c = 2 * b + cc
                            nc.sync.dma_start(
                                out=xt.rearrange("p (kc f) -> p kc f", kc=4),
                                in_=xT[:].transpose([1, 0, 2])[
                                    :, :, c * 512 : (c + 1) * 512
                                ],
                            )
                            xts[cc] = xt
                        return f

                    def mm_part(cc, w_sb, b_sb, dst):
                        def f():
                            ps = ps_qkv.tile([128, 512], FP32, tag="qkv")
                            xt = xts[cc]
                            for kc in range(4):
                                nc.tensor.matmul(
                                    ps[:],
                                    w_sb[:, kc * 128 : (kc + 1) * 128],
                                    xt[:, kc * 512 : (kc + 1) * 512],
                                    start=(kc == 0), stop=(kc == 3),
                                )
                            nc.scalar.add(dst, ps[:], b_sb[:])
                        return f

                    def v_transp(jlo, jhi):
                        def f():
                            for j in range(jlo, jhi):
                                nc.sync.dma_start_transpose(
                                    out=v_nat[:, j * 128 : (j + 1) * 128],
                                    in_=vT_stg[:, j * 128 : (j + 1) * 128],
                                )
                        return f

                    for cc in range(2):
                        c = 2 * b + cc
                        units.append(load_xt(cc))
                        units.append(mm_part(
                            cc, wq_sb, bq_sb, qT_sb[:, c * 512 : (c + 1) * 512]))
                        units.append(mm_part(
                            cc, wk_sb, bk_sb, kT_sb[:, c * 512 : (c + 1) * 512]))
                        units.append(mm_part(
                            cc, wv_sb, bv_sb, vT_stg[:, cc * 512 : (cc + 1) * 512]))
                        units.append(v_transp(4 * cc, 4 * cc + 4))
                    return units

                def proj_units(b):
                    """Normalize + project + store batch b (y_sb/r ready)."""
                    units = []
                    y_sb = state.pop(("y", b))
                    r_sb = state.pop(("r", b))

                    def r_store():
                        for hh in range(4):
                            nc.gpsimd.dma_start(
                                out=den_scr[b * 4 + hh : b * 4 + hh + 1, :],
                                in_=r_sb[32 * hh : 32 * hh + 1, :],
                            )
                    units.append(r_store)

                    r_bc = rpool.tile([128, 1024], FP16, tag="r_bc")

                    def r_load():
                        for hh in range(4):
                            nc.gpsimd.dma_start(
                                out=r_bc[32 * hh : 32 * hh + 32, :],
                                in_=bass.AP(
                                    tensor=den_scr,
                                    offset=(b * 4 + hh) * N,
                                    ap=[[0, 32], [1, N]],
                                ),
                            )
                    units.append(r_load)

                    y_n = ypool.tile([128, 1024], FP16, tag="y_n")

                    def norm():
                        nc.gpsimd.tensor_mul(y_n[:], y_sb[:], r_bc[:])
                    units.append(norm)

                    def pj_cs(cs):
                        def f():
                            o_t = opool.tile([128, 1024], FP16, tag="o_t")
                            for half in range(2):
                                pj = ps_pj.tile([128, 512], FP32, tag="pj")
                                nc.tensor.matmul(
                                    pj[:],
                                    projT_sb[:, cs * 128 : (cs + 1) * 128],
                                    y_n[:, half * 512 : (half + 1) * 512],
                                    start=True, stop=True,
                                )
                                nc.scalar.copy(
                                    o_t[:, half * 512 : (half + 1) * 512], pj[:])
                            nc.sync.dma_start(
                                out=outT[cs * 128 : (cs + 1) * 128,
                                         b * 1024 : (b + 1) * 1024],
                                in_=o_t[:],
                            )
                        return f
                    for cs in range(4):
                        units.append(pj_cs(cs))
                    return units

                def attn_batch(b, woven):
                    """Attention for batch b; `woven` units interleaved into
                    the j-loop to keep PE/DMA fed without starving ACT."""
                    v_nat = state[("vnat", b)]
                    y_sb = ypool.tile([128, 1024], FP16, tag="y_sb")
                    r_sb = rpool.tile([128, 1024], FP32, tag="r_sb")
                    state[("y", b)] = y_sb
                    state[("r", b)] = r_sb
                    wi = 0
                    for half in range(2):
                        av = ps_av.tile([128, 512], FP32, tag="av")
                        den = ps_den.tile([128, 512], FP32, tag="den")

                        def av_den(j, e2s):
                            # AV + den, 4 heads col-packed
                            for hh in range(4):
                                nc.tensor.matmul(
                                    av[32 * hh : 32 * hh + 32, :],
                                    v_nat[:, j * 128 + 32 * hh :
                                          j * 128 + 32 * hh + 32],
                                    e2s[hh // 2][:, (hh % 2) * 512 :
                                                 (hh % 2) * 512 + 512],
                                    start=(j == 0), stop=(j == 7),
                                    tile_position=(0, 32 * hh),
                                )
                            for hh in range(4):
                                nc.tensor.matmul(
                                    den[32 * hh : 32 * hh + 1, :],
                                    ones_sb[:],
                                    e2s[hh // 2][:, (hh % 2) * 512 :
                                                 (hh % 2) * 512 + 512],
                                    start=(j == 0), stop=(j == 7),
                                    tile_position=(0, 32 * hh),
                                )

                        prev = None
                        for j in range(8):
                            path = PATTERN[half * 8 + j]
                            yr0 = 28 - 4 * j + 16 * half
                            # scores: 4-way row-packed K=32 matmuls; for the
                            # D path the bias lands first via diag-identity
                            # matmuls (disjoint 32-partition blocks, each
                            # start=True), then scores accumulate on top
                            s_ts = []
                            for hp in range(2):
                                sps = ps_s.tile([128, 1024], FP32, tag="sps")
                                if path == "D":
                                    for h2 in range(2):
                                        hh = 2 * hp + h2
                                        for u in range(4):
                                            nc.tensor.matmul(
                                                sps[32 * u : 32 * u + 32,
                                                    h2 * 512 : (h2 + 1) * 512],
                                                ident_sb[32 * u : 32 * u + 32, :],
                                                tabD4[32 * u : 32 * u + 32,
                                                      hh, yr0 : yr0 + 16, :],
                                                start=True, stop=False,
                                                tile_position=(32 * u, 32 * u),
                                            )
                                for h2 in range(2):
                                    hh = 2 * hp + h2
                                    nc.tensor.matmul(
                                        sps[:, h2 * 512 : (h2 + 1) * 512],
                                        kT_sb[32 * hh : 32 * hh + 32,
                                              b * 1024 + j * 128 :
                                              b * 1024 + j * 128 + 128],
                                        qT_sb[32 * hh : 32 * hh + 32,
                                              b * 1024 + half * 512 :
                                              b * 1024 + half * 512 + 512],
                                        start=(path != "D"), stop=True,
                                        tile_position=(32 * hh, 0),
                                    )
                                s_ts.append(sps)
                            # exp + bias multiply (path per PATTERN slot)
                            e2s = []
                            for hp in range(2):
                                if path == "C":
                                    # fused exp(S+bias) via int16 bit-trick
                                    e2i = e2pool.tile([128, 1024],
                                                      mybir.dt.int16,
                                                      tag="E2i")
                                    nc.vector.scalar_tensor_tensor(
                                        e2i.rearrange(
                                            "p (h a b) -> p h a b", h=2, a=16),
                                        s_ts[hp].rearrange(
                                            "p (h a b) -> p h a b", h=2, a=16),
                                        A16,
                                        tabL4[:, 2 * hp : 2 * hp + 2,
                                              yr0 : yr0 + 16, :],
                                        mybir.AluOpType.mult,
                                        mybir.AluOpType.add,
                                    )
                                    e2s.append(e2i.bitcast(FP16))
                                    continue
                                e_t = epool.tile([128, 1024], FP16, tag="E")
                                nc.scalar.activation(e_t[:], s_ts[hp][:], EXP)
                                if path == "D":
                                    e2s.append(e_t)
                                    continue
                                e2 = e2pool.tile([128, 1024], FP16, tag="E2")
                                eng = nc.gpsimd if path == "B" else nc.vector
                                eng.tensor_mul(
                                    e2.rearrange("p (h a b) -> p h a b",
                                                 h=2, a=16),
                                    e_t.rearrange("p (h a b) -> p h a b",
                                                  h=2, a=16),
                                    tab4[:, 2 * hp : 2 * hp + 2,
                                         yr0 : yr0 + 16, :],
                                )
                                e2s.append(e2)
                            # lag-1: AV/den for j-1 emit after scores(j) so
                            # the in-order PE queue issues scores(j) without
                            # waiting on the DVE bias-multiply of j-1
                            if prev is not None:
                                av_den(*prev)
                            prev = (j, e2s)
                            if wi < len(woven) and \
                                    wi * 16 < (j + 1 + half * 8) * len(woven):
                                woven[wi]()
                                wi += 1
                        av_den(*prev)
                        nc.scalar.copy(
                            y_sb[:, half * 512 : (half + 1) * 512], av[:])
                        nc.vector.reciprocal_approx_fast(
                            r_sb[:, half * 512 : (half + 1) * 512], den[:])
                    while wi < len(woven):
                        woven[wi]()
                        wi += 1

                # ---- schedule -------------------------------------------
                for u in qkv_units(0):
                    u()
                for b in range(NB):
                    woven = []
                    if b + 1 < NB:
                        woven += qkv_units(b + 1)
                    if b - 1 >= 0:
                        woven += proj_units(b - 1)
                    attn_batch(b, woven)
                for u in proj_units(NB - 1):
                    u()
    nc.compile()
    return nc


def _prep_inputs(x, qkv_w, qkv_b, rpb, proj_w, proj_b):
    x = np.asarray(x, np.float32)
    qkv_w = np.asarray(qkv_w, np.float32)
    qkv_b = np.asarray(qkv_b, np.float32)
    rpb = np.asarray(rpb, np.float32)
    proj_w = np.asarray(proj_w, np.float32)

    p = np.arange(128)
    pm, pd = p % 32, p // 32
    yr = np.arange(60)
    j1 = np.arange(32)
    row = 59 - yr[None, :, None] + pd[:, None, None]      # (128, 60, 1)
    col = 31 + pm[:, None, None] - j1[None, None, :]      # (128, 1, 32)
    row_b = np.broadcast_to(row, (128, 60, 32))
    col_b = np.broadcast_to(col, (128, 60, 32))

    in_maps = []
    for core in range(N_CORES):
        hg, bg = core // 2, core % 2
        xs = x[4 * bg : 4 * bg + 4].reshape(TC, C)
        xT16 = np.ascontiguousarray(xs.T).astype(np.float16).reshape(4, 128, TC)

        r0 = 128 * hg
        wq_in = (qkv_w[r0 : r0 + 128, :] * SCALE).T.reshape(4, 128, 128)
        wk_in = qkv_w[C + r0 : C + r0 + 128, :].T.reshape(4, 128, 128)
        wv_in = qkv_w[2 * C + r0 : 2 * C + r0 + 128, :].T.reshape(4, 128, 128)
        bq_in = (qkv_b[r0 : r0 + 128] * SCALE).astype(np.float32).reshape(128, 1)
        bk_in = qkv_b[C + r0 : C + r0 + 128].astype(np.float32).reshape(128, 1)
        bv_in = qkv_b[2 * C + r0 : 2 * C + r0 + 128].astype(np.float32).reshape(128, 1)

        tabs, tabsL, tabsD = [], [], []
        for hh in range(4):
            e = np.exp(rpb[4 * hg + hh])[row_b, col_b]    # (128, 60, 32)
            tabs.append(e.reshape(128, 1920))
            el = (A16 * rpb[4 * hg + hh] + B16)[row_b, col_b]
            tabsL.append(el.reshape(128, 1920))
            tabsD.append(rpb[4 * hg + hh][row_b, col_b].reshape(128, 1920))
        tab_in = np.concatenate(tabs, axis=1).astype(np.float16)
        tabL_in = np.concatenate(tabsL, axis=1).astype(np.float16)
        tabD_in = np.concatenate(tabsD, axis=1).astype(np.float16)
        ident_in = (np.arange(128)[:, None] % 32 ==
                    np.arange(32)[None, :]).astype(np.float16)

        projT_in = np.ascontiguousarray(
            proj_w[:, r0 : r0 + 128].T).astype(np.float16)

        in_maps.append({
            "xT": xT16,
            "wq": np.ascontiguousarray(wq_in).astype(np.float16),
            "wk": np.ascontiguousarray(wk_in).astype(np.float16),
            "wv": np.ascontiguousarray(wv_in).astype(np.float16),
            "bq": bq_in, "bk": bk_in, "bv": bv_in,
            "tabR": tab_in,
            "tabL": tabL_in,
            "tabD": tabD_in,
            "ident": ident_in,
            "projT": projT_in,
        })
    return in_maps


def kernel(x, qkv_w, qkv_b, rpb, proj_w, proj_b):
    if "nc" not in _CACHE:
        _CACHE["nc"] = build_nc()
    nc = _CACHE["nc"]
    in_maps = _prep_inputs(x, qkv_w, qkv_b, rpb, proj_w, proj_b)
    res = run_bass_kernel_spmd(nc, in_maps, list(range(N_CORES)))
    out = np.zeros((B * N, C), np.float32)
    for core in range(N_CORES):
        hg, bg = core // 2, core % 2
        pt = res.results[core]["outT"].astype(np.float32)   # (512, 4096)
        out[4 * bg * N : (4 * bg + 4) * N, :] += pt.T
    out += np.asarray(proj_b, np.float32)[None, :]
    return out.reshape(B, S, S, C)


if __name__ == "__main__":
    rng = np.random.default_rng(0)
    ins = {
        "x": rng.standard_normal((B, S, S, C)).astype(np.float32),
        "qkv_w": (rng.standard_normal((3 * C, C)) * 0.02).astype(np.float32),
        "qkv_b": (rng.standard_normal((3 * C,)) * 0.02).astype(np.float32),
        "rpb": (rng.standard_normal((NH, 2 * S - 1, 2 * S - 1)) * 0.02).astype(np.float32),
        "proj_w": (rng.standard_normal((C, C)) * 0.02).astype(np.float32),
        "proj_b": (rng.standard_normal((C,)) * 0.02).astype(np.float32),
    }
    out = kernel(**ins)
    print("kernel ran, out", out.shape, out.dtype, float(np.abs(out).max()))
